# revision 15
# baseline (speedup 1.0000x reference)
"""Trainium2 Bass kernel for nn_DeepCFRModel (DeepCFR poker net).

Strategy: pure data-parallel over 8 NeuronCores (4096 rows each, 32 tiles of
128 rows on the partition dim). Host precomputes per-token-position embedding
tables (card/PE folded, QKV projected, biases folded) so all gathers become
one-hot matmuls on the PE. Attention head-reductions run on PE via stationary
elementwise-product tiles x block-ones; AV j-reduction via identity-matmul
PSUM accumulation. LayerNorm stats via bn_stats on row-major tiles.

All PE operands are bf16 (fp32 matmuls pay a ~4x stationary-load penalty);
PSUM accumulation stays fp32, as do LayerNorm stats, softmax, and the GRU
state arithmetic.
"""
import sys

if "/opt/trn_rl_repo" not in sys.path:
    sys.path.insert(0, "/opt/trn_rl_repo")

import numpy as np
import ml_dtypes

import concourse.bass as bass
import concourse.bacc as bacc
import concourse.mybir as mybir
import concourse.tile as tile
from concourse import bass_utils
from concourse.bass import ds, ts

F32 = mybir.dt.float32
BF16 = mybir.dt.bfloat16
AF = mybir.ActivationFunctionType
ALU = mybir.AluOpType
AX = mybir.AxisListType

P = 128
N_CORES = 8
B = 32768
D = 256
NH = 8
HD = 32
H = 256
EPS = 1e-5
BF = ml_dtypes.bfloat16


# ---------------------------------------------------------------- host prep

def _pe_table():
    pos = np.arange(5, dtype=np.float32)[:, None]
    div = np.exp(np.arange(0, D, 2, dtype=np.float32) * (-np.log(10000.0) / D))
    pe = np.zeros((5, D), np.float32)
    pe[:, 0::2] = np.sin(pos * div)
    pe[:, 1::2] = np.cos(pos * div)
    return pe


def _prep_consts(params):
    """Host-side packing of all weights/tables. Returns dict name -> np array
    (these become ExternalInputs, identical on every core) plus flag dict."""
    p = {k: np.asarray(v, np.float32) for k, v in params.items()}
    c = {}
    fl = {}

    # ---- token tables [53, 7, 256] (partition = token id) ----
    ct = p["card_table"]
    pe = _pe_table()
    xt = np.zeros((7, 53, D), np.float32)
    for t in range(7):
        xt[t, 1:] = ct
        if t >= 2:
            xt[t] += pe[t - 2][None, :]
    qt = (xt @ p["Wq"] + p["bq"]) / np.sqrt(HD)
    kt = xt @ p["Wk"] + p["bk"]
    vt = xt @ p["Wv"] + p["bv"]
    c["xtab"] = np.ascontiguousarray(xt.transpose(1, 0, 2)).astype(BF)
    c["qtab"] = np.ascontiguousarray(qt.transpose(1, 0, 2)).astype(BF)
    c["ktab"] = np.ascontiguousarray(kt.transpose(1, 0, 2)).astype(BF)
    c["vtab"] = np.ascontiguousarray(vt.transpose(1, 0, 2)).astype(BF)

    # ---- attention misc ----
    c["Wo"] = np.ascontiguousarray(
        p["Wo"].reshape(2, 128, 256).transpose(1, 0, 2)).astype(BF)
    fl["bo"] = bool(np.any(p["bo"]))
    c["bo"] = p["bo"].reshape(1, 256)
    fl["attn_gb"] = bool(np.any(p["attn_g"] != 1.0) or np.any(p["attn_b"]))
    c["attn_g"] = np.broadcast_to(p["attn_g"], (P, 256)).copy()
    c["attn_b"] = np.broadcast_to(p["attn_b"], (P, 256)).copy()

    # ---- f1 ----
    f1W = p["f1W"]
    c["f1Wc"] = np.ascontiguousarray(
        f1W[:1792].reshape(14, 128, 1024).transpose(1, 0, 2)).astype(BF)
    c["f1W9"] = f1W[1792:1801].astype(BF)                      # [9,1024]
    c["stage_f1"] = (p["stage_emb"] @ f1W[1801:1833]).astype(BF)   # [4,1024]
    c["pos_f1"] = (p["pos_emb"] @ f1W[1833:1865]).astype(BF)       # [3,1024]
    fl["f1b"] = bool(np.any(p["f1b"]))
    c["f1b"] = p["f1b"].reshape(1, 1024)
    fl["f1gb"] = bool(np.any(p["f1g"] != 1.0) or np.any(p["f1beta"]))
    c["f1g"] = np.broadcast_to(p["f1g"], (P, 1024)).copy()
    c["f1beta"] = np.broadcast_to(p["f1beta"], (P, 1024)).copy()

    # ---- f2, f3 ----
    c["f2W"] = np.ascontiguousarray(
        p["f2W"].reshape(8, 128, 512).transpose(1, 0, 2)).astype(BF)
    fl["f2b"] = bool(np.any(p["f2b"]))
    c["f2b"] = p["f2b"].reshape(1, 512)
    fl["f2gb"] = bool(np.any(p["f2g"] != 1.0) or np.any(p["f2beta"]))
    c["f2g"] = np.broadcast_to(p["f2g"], (P, 512)).copy()
    c["f2beta"] = np.broadcast_to(p["f2beta"], (P, 512)).copy()

    c["f3W"] = np.ascontiguousarray(
        p["f3W"].reshape(4, 128, 256).transpose(1, 0, 2)).astype(BF)
    fl["f3b"] = bool(np.any(p["f3b"]))
    c["f3b"] = p["f3b"].reshape(1, 256)
    fl["f3gb"] = bool(np.any(p["f3g"] != 1.0) or np.any(p["f3beta"]))
    c["f3g"] = np.broadcast_to(p["f3g"], (P, 256)).copy()
    c["f3beta"] = np.broadcast_to(p["f3beta"], (P, 256)).copy()

    # ---- GRU ----
    Wzr = np.concatenate([p["Wz"], p["Wr"]], axis=1)           # [512,512]
    c["Wzr"] = np.ascontiguousarray(
        Wzr.reshape(4, 128, 512).transpose(1, 0, 2)).astype(BF)
    fl["bzr"] = bool(np.any(p["bz"]) or np.any(p["br"]))
    c["bzr"] = np.concatenate([p["bz"], p["br"]]).reshape(1, 512)
    c["Wc"] = np.ascontiguousarray(
        p["Wc"].reshape(4, 128, 256).transpose(1, 0, 2)).astype(BF)
    fl["bc"] = bool(np.any(p["bc"]))
    c["bc"] = p["bc"].reshape(1, 256)

    # ---- main net ----
    m1Wf = p["m1W"] + p["m1skip"]
    c["m1W"] = np.ascontiguousarray(
        m1Wf.reshape(4, 128, 256).transpose(1, 0, 2)).astype(BF)
    fl["m1b"] = bool(np.any(p["m1b"]))
    c["m1b"] = p["m1b"].reshape(1, 256)
    fl["m1gb"] = bool(np.any(p["m1g"] != 1.0) or np.any(p["m1beta"]))
    c["m1g"] = np.broadcast_to(p["m1g"], (P, 256)).copy()
    c["m1beta"] = np.broadcast_to(p["m1beta"], (P, 256)).copy()

    c["m2W"] = np.ascontiguousarray(
        p["m2W"].reshape(2, 128, 256).transpose(1, 0, 2)).astype(BF)
    fl["m2b"] = bool(np.any(p["m2b"]))
    c["m2b"] = p["m2b"].reshape(1, 256)
    fl["m2gb"] = bool(np.any(p["m2g"] != 1.0) or np.any(p["m2beta"]))
    c["m2g"] = np.broadcast_to(p["m2g"], (P, 256)).copy()
    c["m2beta"] = np.broadcast_to(p["m2beta"], (P, 256)).copy()

    # ---- heads ----
    hW1 = p["hW1"]                                             # [4,256,128]
    hW1a = np.concatenate([hW1[e] for e in range(4)], axis=1)  # [256,512]
    c["hW1a"] = np.ascontiguousarray(
        hW1a.reshape(2, 128, 512).transpose(1, 0, 2)).astype(BF)
    fl["hb1"] = bool(np.any(p["hb1"]))
    hb1b = np.zeros((4, 512), np.float32)
    for e in range(4):
        hb1b[e, e * 128:(e + 1) * 128] = p["hb1"][e]
    c["hb1b"] = hb1b.astype(BF)
    c["hW2"] = np.ascontiguousarray(p["hW2"].transpose(1, 0, 2)).astype(BF)
    fl["hb2"] = bool(np.any(p["hb2"]))
    c["hb2"] = p["hb2"]                                        # [4,10] f32

    # ---- constants ----
    c["iota53"] = np.broadcast_to(np.arange(53, dtype=np.float32), (P, 53)).copy()
    c["iota4"] = np.broadcast_to(np.arange(4, dtype=np.float32), (P, 4)).copy()
    c["iota3"] = np.broadcast_to(np.arange(3, dtype=np.float32), (P, 3)).copy()
    c["identb"] = np.eye(P, dtype=np.float32).astype(BF)
    bones = np.zeros((128, 2, 8), np.float32)
    for half in range(2):
        for r in range(128):
            bones[r, half, (half * 128 + r) // 32] = 1.0
    c["bones"] = bones.astype(BF)
    c["ones1"] = np.ones((1, P), np.float32).astype(BF)
    c["epsv"] = np.full((P, 1), EPS, np.float32)

    # drop consts that the trivial-parameter specialization never reads
    drop = []
    if not fl["bo"]:
        drop.append("bo")
    if not fl["attn_gb"]:
        drop += ["attn_g", "attn_b"]
    for pref in ("f1", "f2", "f3", "m1", "m2"):
        if not fl[pref + "b"]:
            drop.append(pref + "b")
        if not fl[pref + "gb"]:
            drop += [pref + "g", pref + "beta"]
    if not fl["bzr"]:
        drop.append("bzr")
    if not fl["bc"]:
        drop.append("bc")
    if not fl["hb1"]:
        drop.append("hb1b")
    if not fl["hb2"]:
        drop.append("hb2")
    if not any(fl[k] for k in ("bo", "f1b", "f2b", "f3b", "bzr", "bc", "m1b", "m2b")):
        drop.append("ones1")
    for k in drop:
        c.pop(k, None)
    return c, fl


def _prep_percore(inputs, bc):
    """Per-core input arrays: misc [bc,28] and prevctx [bc,256]."""
    priv = np.asarray(inputs["private_cards"]).astype(np.float32) + 1.0
    pub = np.asarray(inputs["public_cards"]).astype(np.float32) + 1.0
    ids = np.concatenate([priv, pub], axis=1)                  # [B,7], 0=pad
    stage = np.asarray(inputs["stage"]).astype(np.float32)
    pos = np.asarray(inputs["current_player_pos"]).astype(np.float32)
    scal9 = np.concatenate([np.asarray(inputs["stacks"]),
                            np.asarray(inputs["bets"]),
                            np.asarray(inputs["active_players_mask"])],
                           axis=1).astype(np.float32)
    amask = np.asarray(inputs["actions_mask"]).astype(np.float32)
    misc = np.concatenate([ids, stage, pos, scal9, amask], axis=1)  # [B,28]
    pctx = np.asarray(inputs["prev_context"]).astype(np.float32)
    maps = []
    for cc in range(N_CORES):
        sl = slice(cc * bc, (cc + 1) * bc)
        maps.append({"misc": np.ascontiguousarray(misc[sl]),
                     "prevctx": np.ascontiguousarray(pctx[sl])})
    return maps


# ---------------------------------------------------------------- kernel build

def build_kernel(n_tiles, consts, fl):
    bc = n_tiles * P
    nc = bacc.Bacc("TRN2", target_bir_lowering=False, debug=False,
                   num_devices=N_CORES)

    dr = {}
    for name, arr in consts.items():
        dt = BF16 if arr.dtype == BF else F32
        dr[name] = nc.dram_tensor(name, list(arr.shape), dt, kind="ExternalInput")
    misc_d = nc.dram_tensor("misc", [bc, 28], F32, kind="ExternalInput")
    pctx_d = nc.dram_tensor("prevctx", [bc, 256], F32, kind="ExternalInput")
    probs_d = nc.dram_tensor("probs_out", [bc, 10], F32, kind="ExternalOutput")
    nctx_d = nc.dram_tensor("nctx_out", [bc, 256], F32, kind="ExternalOutput")

    with tile.TileContext(nc) as tc:
        _emit(nc, tc, dr, fl, misc_d, pctx_d, probs_d, nctx_d, n_tiles)
    nc.compile()
    return nc


def _emit(nc, tc, dr, fl, misc_d, pctx_d, probs_d, nctx_d, n_tiles):
    import contextlib
    ctx = contextlib.ExitStack()
    with ctx:
        # ---------------- weight pools (resident, bufs=1) ----------------
        wp = ctx.enter_context(tc.tile_pool(name="weights", bufs=1))
        W = {}
        for name, d in dr.items():
            t = wp.tile(list(d.shape), d.dtype, tag=name, name=name)
            nc.sync.dma_start(t[...], d[...])
            W[name] = t

        # ---------------- transient pools ----------------
        io = ctx.enter_context(tc.tile_pool(name="io", bufs=2))
        sm = ctx.enter_context(tc.tile_pool(name="small", bufs=2))
        mid = ctx.enter_context(tc.tile_pool(name="mid", bufs=2))
        mid2 = ctx.enter_context(tc.tile_pool(name="mid2", bufs=2))
        big = ctx.enter_context(tc.tile_pool(name="big", bufs=2))
        tpo = ctx.enter_context(tc.tile_pool(name="tpose", bufs=2))

        psBig = ctx.enter_context(tc.tile_pool(name="psBig", bufs=2, space="PSUM"))
        psS = ctx.enter_context(tc.tile_pool(name="psS", bufs=1, space="PSUM"))
        psM = ctx.enter_context(tc.tile_pool(name="psM", bufs=3, space="PSUM"))

        identb = W["identb"]

        def transpose_pack(out_sb, srcs):
            """PE-transpose each [128,128-or-less] bf16 src into one shared
            PSUM tile (all fit one bank), then one ACT copy to out_sb (bf16).
            srcs: list of APs, each [128, w]; out_sb [sum-w?, k, 128]-ish AP
            arranged [w, k, 128] written slice per src."""
            k = len(srcs)
            w = max(s.shape[1] for s in srcs)
            pst = psM.tile([w, k, P], BF16, tag="m", name="pstT")
            for idx, s in enumerate(srcs):
                nc.tensor.matmul(pst[0:s.shape[1], idx, :], s, identb[...],
                                 is_transpose=True, start=True, stop=True)
            nc.scalar.copy(out_sb, pst[...])

        def ln_apply(z_ps, width, pref, gelu, out_tile):
            """Full-width LN over `width` (+ optional gelu) PSUM z -> SBUF."""
            csz = min(width, 512)
            chunks = width // csz
            stats = sm.tile([P, chunks, 6], F32, tag="stats", name="stats")
            zc = z_ps.rearrange("p (c f) -> p c f", c=chunks)
            for ci in range(chunks):
                nc.vector.bn_stats(stats[:, ci, :], zc[:, ci, :])
            mv = sm.tile([P, 2], F32, tag="mv", name="mv")
            nc.vector.bn_aggr(mv[...], stats[...])
            sd = sm.tile([P, 1], F32, tag="sd", name="sd")
            nc.scalar.activation(sd[...], mv[:, 1:2], AF.Sqrt, bias=W["epsv"][...])
            rstd = sm.tile([P, 1], F32, tag="rstd", name="rstd")
            nc.vector.reciprocal(rstd[...], sd[...])
            if fl[pref + "gb"]:
                t0 = mid.tile([P, width], F32, tag="lnt0", name="lnt0")
                nc.vector.tensor_scalar(t0[...], z_ps, mv[:, 0:1], rstd[...],
                                        ALU.subtract, ALU.mult)
                t1 = mid.tile([P, width], F32, tag="lnt1", name="lnt1")
                nc.vector.tensor_tensor(t1[...], t0[...], W[pref + "g"][...], ALU.mult)
                pre = mid.tile([P, width], F32, tag="lnt2", name="lnt2")
                nc.vector.tensor_tensor(pre[...], t1[...], W[pref + "beta"][...], ALU.add)
            else:
                pre = mid.tile([P, width], F32, tag="lnt0", name="lnt0")
                nc.vector.tensor_scalar(pre[...], z_ps, mv[:, 0:1], rstd[...],
                                        ALU.subtract, ALU.mult)
            if gelu:
                nc.scalar.activation(out_tile[...], pre[...], AF.Gelu)
            else:
                nc.scalar.copy(out_tile[...], pre[...])

        def ln_apply_2ps(z_halves, width, pref, gelu, out_tile):
            """LN over `width` spread across two PSUM half-tiles."""
            hw = width // 2
            chunks_per = max(1, hw // 512)
            csz = hw // chunks_per
            nch = 2 * chunks_per
            stats = sm.tile([P, nch, 6], F32, tag="stats", name="stats2")
            for hi in range(2):
                zc = z_halves[hi][...].rearrange("p (c f) -> p c f", c=chunks_per)
                for ci in range(chunks_per):
                    nc.vector.bn_stats(stats[:, hi * chunks_per + ci, :], zc[:, ci, :])
            mv = sm.tile([P, 2], F32, tag="mv", name="mv2")
            nc.vector.bn_aggr(mv[...], stats[...])
            sd = sm.tile([P, 1], F32, tag="sd", name="sd2")
            nc.scalar.activation(sd[...], mv[:, 1:2], AF.Sqrt, bias=W["epsv"][...])
            rstd = sm.tile([P, 1], F32, tag="rstd", name="rstd2")
            nc.vector.reciprocal(rstd[...], sd[...])
            for hi in range(2):
                if fl[pref + "gb"]:
                    t0 = mid.tile([P, hw], F32, tag="lnt0", name="lnt0h")
                    nc.vector.tensor_scalar(t0[...], z_halves[hi][...],
                                            mv[:, 0:1], rstd[...],
                                            ALU.subtract, ALU.mult)
                    t1 = mid.tile([P, hw], F32, tag="lnt1", name="lnt1h")
                    nc.vector.tensor_tensor(
                        t1[...], t0[...], W[pref + "g"][:, ds(hi * hw, hw)], ALU.mult)
                    pre = mid.tile([P, hw], F32, tag="lnt2", name="lnt2h")
                    nc.vector.tensor_tensor(
                        pre[...], t1[...], W[pref + "beta"][:, ds(hi * hw, hw)], ALU.add)
                else:
                    pre = mid.tile([P, hw], F32, tag="lnt0", name="lnt0h")
                    nc.vector.tensor_scalar(pre[...], z_halves[hi][...],
                                            mv[:, 0:1], rstd[...],
                                            ALU.subtract, ALU.mult)
                if gelu:
                    nc.scalar.activation(out_tile[:, ds(hi * hw, hw)], pre[...],
                                         AF.Gelu)
                else:
                    nc.scalar.copy(out_tile[:, ds(hi * hw, hw)], pre[...])

        def bias_mm(ps_ap, bias_name, n0, n1, stop):
            nc.tensor.matmul(ps_ap, W["ones1"][0:1, :], W[bias_name][0:1, n0:n1],
                             start=False, stop=stop)

        # ================= per-tile loop =================
        import contextlib as _cl
        for t in range(n_tiles):
            _ts = ctx.enter_context if False else None
            tile_scope = nc.named_scope(f"tile{t:02d}")
            tile_scope.__enter__()
            # ---- input DMA ----
            misc = io.tile([P, 28], F32, tag="misc", name="misc")
            nc.sync.dma_start(misc[...], misc_d[ts(t, P), :])
            pctx = io.tile([P, 256], F32, tag="pctx", name="pctx")
            nc.sync.dma_start(pctx[...], pctx_d[ts(t, P), :])
            ids = misc[:, 0:7]
            stagef = misc[:, 7:8]
            posf = misc[:, 8:9]
            scal9 = misc[:, 9:18]
            amask = misc[:, 18:28]

            # ---- one-hots (DVE; bf16 out, values 0/1 exact) ----
            oh = mid2.tile([P, 7, 53], BF16, tag="oh", name="oh")
            nc.vector.tensor_tensor(
                oh[...], ids[:, :, None].broadcast_to([P, 7, 53]),
                W["iota53"][:, None, :].broadcast_to([P, 7, 53]), ALU.is_equal)
            oh4 = sm.tile([P, 4], BF16, tag="oh4", name="oh4")
            nc.vector.tensor_tensor(oh4[...], stagef.broadcast_to([P, 4]),
                                    W["iota4"][...], ALU.is_equal)
            oh4f = sm.tile([P, 4], F32, tag="oh4f", name="oh4f")
            nc.vector.tensor_tensor(oh4f[...], stagef.broadcast_to([P, 4]),
                                    W["iota4"][...], ALU.is_equal)
            oh3 = sm.tile([P, 3], BF16, tag="oh3", name="oh3")
            nc.vector.tensor_tensor(oh3[...], posf.broadcast_to([P, 3]),
                                    W["iota3"][...], ALU.is_equal)
            s9b = sm.tile([P, 9], BF16, tag="s9b", name="s9b")
            nc.scalar.copy(s9b[...], scal9)
            pctxb = sm.tile([P, 256], BF16, tag="pctxb", name="pctxb")
            nc.scalar.copy(pctxb[...], pctx[...])
            mb = sm.tile([P, 7], F32, tag="mb", name="mb")
            nc.vector.tensor_scalar(mb[...], ids, 0.0, -1e9, ALU.is_equal, ALU.mult)

            # ---- transposes of onehots / small inputs (packed) ----
            ohT = mid2.tile([53, 7, P], BF16, tag="ohT", name="ohT")
            transpose_pack(ohT[:, 0:4, :], [oh[:, tok, :] for tok in range(4)])
            transpose_pack(ohT[:, 4:7, :], [oh[:, tok, :] for tok in range(4, 7)])
            smT = tpo.tile([9, 3, P], BF16, tag="smT", name="smT")
            transpose_pack(smT[...], [oh4[...], oh3[...], s9b[...]])
            ohT4, ohT3, s9T = smT[0:4, 0, :], smT[0:3, 1, :], smT[0:9, 2, :]
            pctxT = tpo.tile([P, 2, P], BF16, tag="pctxT", name="pctxT")
            transpose_pack(pctxT[...], [pctxb[:, ds(c * P, P)] for c in range(2)])

            # ---- phase A: q,k FM halves + scores (accumulate across halves) ----
            sp = psS.tile([P, 7, 7, 8], F32, tag="s", name="sp")
            for half in range(2):
                qp = psBig.tile([P, 7, P], F32, tag="big", name="qp")
                kp = psBig.tile([P, 7, P], F32, tag="big", name="kp")
                for tok in range(7):
                    nc.tensor.matmul(qp[:, tok, :],
                                     W["qtab"][:, tok, ds(half * P, P)],
                                     ohT[:, tok, :], start=True, stop=True)
                    nc.tensor.matmul(kp[:, tok, :],
                                     W["ktab"][:, tok, ds(half * P, P)],
                                     ohT[:, tok, :], start=True, stop=True)
                q_sb = mid2.tile([P, 7, P], BF16, tag="q_sb", name="q_sb")
                nc.scalar.copy(q_sb[...], qp[...])
                k_sb = mid2.tile([P, 7, P], BF16, tag="k_sb", name="k_sb")
                nc.scalar.copy(k_sb[...], kp[...])
                for i in range(7):
                    e_sb = mid2.tile([P, 7, P], BF16, tag="e", name="e")
                    nc.vector.tensor_tensor(
                        e_sb[...], q_sb[:, i:i + 1, :].broadcast_to([P, 7, P]),
                        k_sb[...], ALU.mult)
                    for j in range(7):
                        nc.tensor.matmul(sp[:, i, j, :], e_sb[:, j, :],
                                         W["bones"][:, half, :],
                                         start=(half == 0), stop=(half == 1))

            # ---- softmax (row-major) ----
            smx = mid.tile([P, 7, 7, 8], F32, tag="smx", name="smx")
            nc.vector.tensor_tensor(
                smx[...], sp[...],
                mb[:, None, :, None].broadcast_to([P, 7, 7, 8]), ALU.add)
            pex = mid.tile([P, 7, 7, 8], F32, tag="pex", name="pex")
            nc.scalar.activation(pex[...], smx[...], AF.Exp)
            zs = sm.tile([P, 7, 8], F32, tag="zs", name="zs")
            nc.vector.tensor_reduce(zs[...], pex[...].rearrange("p i j h -> p i h j"),
                                    AX.X, ALU.add)
            zr = sm.tile([P, 7, 8], F32, tag="zr", name="zr")
            nc.vector.reciprocal(zr[...], zs[...])
            pn = mid.tile([P, 7, 7, 8], BF16, tag="pn", name="pn")
            nc.vector.tensor_tensor(
                pn[...], pex[...],
                zr[:, :, None, :].broadcast_to([P, 7, 7, 8]), ALU.mult)

            # ---- phase B: v + AV ----
            v03 = psBig.tile([P, 4, 256], F32, tag="big", name="v03")
            v46 = psBig.tile([P, 3, 256], F32, tag="big", name="v46")
            for tok in range(7):
                vdst = v03[:, tok, :] if tok < 4 else v46[:, tok - 4, :]
                nc.tensor.matmul(vdst, ohT[:, tok, :], W["vtab"][:, tok, :],
                                 start=True, stop=True)
            v_sb = big.tile([P, 7, 256], BF16, tag="v_sb", name="v_sb")
            nc.scalar.copy(v_sb[:, 0:4, :], v03[...])
            nc.scalar.copy(v_sb[:, 4:7, :], v46[...])
            o_sb = big.tile([P, 7, 256], BF16, tag="o", name="o_sb")
            for i in range(7):
                pv = big.tile([P, 7, 8, HD], BF16, tag="pv", name="pv")
                nc.vector.tensor_tensor(
                    pv[...],
                    pn[:, i, :, :, None].broadcast_to([P, 7, 8, HD]),
                    v_sb[...].rearrange("p j (h d) -> p j h d", h=8), ALU.mult)
                op = psM.tile([P, 256], F32, tag="m", name="op")
                for j in range(7):
                    nc.tensor.matmul(op[...], identb[...],
                                     pv[:, j, :, :].rearrange("p h d -> p (h d)"),
                                     start=(j == 0), stop=(j == 6))
                nc.scalar.copy(o_sb[:, i, :], op[...])

            # ---- phase C: o-proj + x + per-token LN ----
            cp03 = psBig.tile([P, 4, 256], F32, tag="big", name="cp03")
            cp46 = psBig.tile([P, 3, 256], F32, tag="big", name="cp46")
            for tok in range(7):
                cdst = cp03[:, tok, :] if tok < 4 else cp46[:, tok - 4, :]
                oT = tpo.tile([P, 2, P], BF16, tag="oT", name="oT")
                transpose_pack(oT[...], [o_sb[:, tok, ds(c * P, P)] for c in range(2)])
                nc.tensor.matmul(cdst, oT[:, 0, :], W["Wo"][:, 0, :],
                                 start=True, stop=False)
                nc.tensor.matmul(cdst, oT[:, 1, :], W["Wo"][:, 1, :],
                                 start=False, stop=False)
                nc.tensor.matmul(cdst, ohT[:, tok, :], W["xtab"][:, tok, :],
                                 start=False, stop=not fl["bo"])
                if fl["bo"]:
                    bias_mm(cdst, "bo", 0, 256, True)

            cf = big.tile([P, 7, 256], BF16, tag="cf", name="cf")
            for grp, cpt, ntok in ((0, cp03, 4), (4, cp46, 3)):
                stats = sm.tile([P, ntok, 6], F32, tag="stats", name="statsA")
                for tt in range(ntok):
                    nc.vector.bn_stats(stats[:, tt, :], cpt[:, tt, :])
                for tt in range(ntok):
                    tok = grp + tt
                    mv = sm.tile([P, 2], F32, tag="mv", name="mvA")
                    nc.vector.bn_aggr(mv[...], stats[:, tt, :])
                    sd = sm.tile([P, 1], F32, tag="sd", name="sdA")
                    nc.scalar.activation(sd[...], mv[:, 1:2], AF.Sqrt,
                                         bias=W["epsv"][...])
                    rstd = sm.tile([P, 1], F32, tag="rstd", name="rstdA")
                    nc.vector.reciprocal(rstd[...], sd[...])
                    if fl["attn_gb"]:
                        cfw = mid.tile([P, 256], F32, tag="cfw", name="cfw")
                        nc.vector.tensor_scalar(cfw[...], cpt[:, tt, :],
                                                mv[:, 0:1], rstd[...],
                                                ALU.subtract, ALU.mult)
                        cfw2 = mid.tile([P, 256], F32, tag="cfw2", name="cfw2")
                        nc.vector.tensor_tensor(cfw2[...], cfw[...],
                                                W["attn_g"][...], ALU.mult)
                        nc.vector.tensor_tensor(cf[:, tok, :], cfw2[...],
                                                W["attn_b"][...], ALU.add)
                    else:
                        nc.vector.tensor_scalar(cf[:, tok, :], cpt[:, tt, :],
                                                mv[:, 0:1], rstd[...],
                                                ALU.subtract, ALU.mult)

            # ---- f1 (two 1-bank PSUM halves) ----
            h1ph = [psM.tile([P, 512], F32, tag="m", name="h1pA"),
                    psM.tile([P, 512], F32, tag="m", name="h1pB")]
            cfv = cf[...].rearrange("p t d -> p (t d)")
            for kb2 in range(7):
                cfT = tpo.tile([P, 2, P], BF16, tag="cfT", name="cfT")
                transpose_pack(cfT[...],
                               [cfv[:, ds((2 * kb2 + c) * P, P)] for c in range(2)])
                for c in range(2):
                    kb = 2 * kb2 + c
                    for nh2 in range(2):
                        nc.tensor.matmul(h1ph[nh2][...], cfT[:, c, :],
                                         W["f1Wc"][:, kb, ds(nh2 * 512, 512)],
                                         start=(kb == 0), stop=False)
            for nh2 in range(2):
                nsl = ds(nh2 * 512, 512)
                nc.tensor.matmul(h1ph[nh2][...], s9T, W["f1W9"][:, nsl],
                                 start=False, stop=False)
                nc.tensor.matmul(h1ph[nh2][...], ohT4, W["stage_f1"][:, nsl],
                                 start=False, stop=False)
                nc.tensor.matmul(h1ph[nh2][...], ohT3, W["pos_f1"][:, nsl],
                                 start=False, stop=not fl["f1b"])
                if fl["f1b"]:
                    bias_mm(h1ph[nh2][...], "f1b", nh2 * 512, nh2 * 512 + 512, True)
            h1 = mid.tile([P, 1024], BF16, tag="h1", name="h1")
            ln_apply_2ps(h1ph, 1024, "f1", True, h1)

            # ---- f2 ----
            h2p = psM.tile([P, 512], F32, tag="m", name="h2p")
            for kb2 in range(4):
                h1T = tpo.tile([P, 2, P], BF16, tag="h1T", name="h1T")
                transpose_pack(h1T[...],
                               [h1[:, ds((2 * kb2 + c) * P, P)] for c in range(2)])
                for c in range(2):
                    kb = 2 * kb2 + c
                    nc.tensor.matmul(h2p[...], h1T[:, c, :], W["f2W"][:, kb, :],
                                     start=(kb == 0),
                                     stop=(kb == 7) and not fl["f2b"])
            if fl["f2b"]:
                bias_mm(h2p[...], "f2b", 0, 512, True)
            h2 = mid.tile([P, 512], BF16, tag="h2", name="h2")
            ln_apply(h2p[...], 512, "f2", True, h2)

            # ---- f3 ----
            f3p = psM.tile([P, 256], F32, tag="m", name="f3p")
            h2T = tpo.tile([P, 4, P], BF16, tag="h2T", name="h2T")
            transpose_pack(h2T[...], [h2[:, ds(c * P, P)] for c in range(4)])
            for kb in range(4):
                nc.tensor.matmul(f3p[...], h2T[:, kb, :], W["f3W"][:, kb, :],
                                 start=(kb == 0), stop=(kb == 3) and not fl["f3b"])
            if fl["f3b"]:
                bias_mm(f3p[...], "f3b", 0, 256, True)
            feats = io.tile([P, 256], BF16, tag="feats", name="feats")
            ln_apply(f3p[...], 256, "f3", True, feats)

            # ---- GRU ----
            featsT = tpo.tile([P, 2, P], BF16, tag="featsT", name="featsT")
            transpose_pack(featsT[...], [feats[:, ds(c * P, P)] for c in range(2)])
            zrp = psM.tile([P, 512], F32, tag="m", name="zrp")
            combT = [featsT[:, 0, :], featsT[:, 1, :], pctxT[:, 0, :], pctxT[:, 1, :]]
            for kb in range(4):
                nc.tensor.matmul(zrp[...], combT[kb], W["Wzr"][:, kb, :],
                                 start=(kb == 0), stop=(kb == 3) and not fl["bzr"])
            if fl["bzr"]:
                bias_mm(zrp[...], "bzr", 0, 512, True)
            zr_sb = mid.tile([P, 512], F32, tag="zrg", name="zr_sb")
            nc.scalar.activation(zr_sb[...], zrp[...], AF.Sigmoid)
            rp = sm.tile([P, 256], BF16, tag="rp", name="rp")
            nc.vector.tensor_tensor(rp[...], zr_sb[:, 256:512], pctx[...], ALU.mult)
            rpT = tpo.tile([P, 2, P], BF16, tag="rpT", name="rpT")
            transpose_pack(rpT[...], [rp[:, ds(c * P, P)] for c in range(2)])
            cdp = psM.tile([P, 256], F32, tag="m", name="cdp")
            candT = [featsT[:, 0, :], featsT[:, 1, :], rpT[:, 0, :], rpT[:, 1, :]]
            for kb in range(4):
                nc.tensor.matmul(cdp[...], candT[kb], W["Wc"][:, kb, :],
                                 start=(kb == 0), stop=(kb == 3) and not fl["bc"])
            if fl["bc"]:
                bias_mm(cdp[...], "bc", 0, 256, True)
            cand = sm.tile([P, 256], F32, tag="cand", name="cand")
            nc.scalar.activation(cand[...], cdp[...], AF.Tanh)
            t1 = sm.tile([P, 256], F32, tag="t1", name="t1")
            nc.vector.tensor_tensor(t1[...], cand[...], pctx[...], ALU.subtract)
            t2 = sm.tile([P, 256], F32, tag="t2", name="t2")
            nc.vector.tensor_tensor(t2[...], zr_sb[:, 0:256], t1[...], ALU.mult)
            nctx = io.tile([P, 256], F32, tag="nctx", name="nctx")
            nc.vector.tensor_tensor(nctx[...], pctx[...], t2[...], ALU.add)
            nc.sync.dma_start(nctx_d[ts(t, P), :], nctx[...])
            nctxb = sm.tile([P, 256], BF16, tag="nctxb", name="nctxb")
            nc.scalar.copy(nctxb[...], nctx[...])

            # ---- m1 ----
            nctxT = tpo.tile([P, 2, P], BF16, tag="nctxT", name="nctxT")
            transpose_pack(nctxT[...], [nctxb[:, ds(c * P, P)] for c in range(2)])
            m1p = psM.tile([P, 256], F32, tag="m", name="m1p")
            mcT = [featsT[:, 0, :], featsT[:, 1, :], nctxT[:, 0, :], nctxT[:, 1, :]]
            for kb in range(4):
                nc.tensor.matmul(m1p[...], mcT[kb], W["m1W"][:, kb, :],
                                 start=(kb == 0), stop=(kb == 3) and not fl["m1b"])
            if fl["m1b"]:
                bias_mm(m1p[...], "m1b", 0, 256, True)
            y1 = sm.tile([P, 256], BF16, tag="y1", name="y1")
            ln_apply(m1p[...], 256, "m1", True, y1)

            # ---- m2 (+residual) ----
            m2p = psM.tile([P, 256], F32, tag="m", name="m2p")
            y1T = tpo.tile([P, 2, P], BF16, tag="y1T", name="y1T")
            transpose_pack(y1T[...], [y1[:, ds(c * P, P)] for c in range(2)])
            for kb in range(2):
                nc.tensor.matmul(m2p[...], y1T[:, kb, :], W["m2W"][:, kb, :],
                                 start=(kb == 0), stop=False)
            nc.tensor.matmul(m2p[...], identb[...], y1[...],
                             start=False, stop=not fl["m2b"])
            if fl["m2b"]:
                bias_mm(m2p[...], "m2b", 0, 256, True)
            y2 = sm.tile([P, 256], BF16, tag="y2", name="y2")
            ln_apply(m2p[...], 256, "m2", True, y2)

            # ---- heads ----
            hhp = psM.tile([P, 512], F32, tag="m", name="hhp")
            y2T = tpo.tile([P, 2, P], BF16, tag="y2T", name="y2T")
            transpose_pack(y2T[...], [y2[:, ds(c * P, P)] for c in range(2)])
            for kb in range(2):
                nc.tensor.matmul(hhp[...], y2T[:, kb, :], W["hW1a"][:, kb, :],
                                 start=(kb == 0), stop=(kb == 1) and not fl["hb1"])
            if fl["hb1"]:
                nc.tensor.matmul(hhp[...], ohT4, W["hb1b"][...],
                                 start=False, stop=True)
            hh = mid.tile([P, 512], BF16, tag="hh", name="hh")
            nc.scalar.activation(hh[...], hhp[...], AF.Gelu)
            hhT = tpo.tile([P, 4, P], BF16, tag="hhT", name="hhT")
            transpose_pack(hhT[...], [hh[:, ds(e * P, P)] for e in range(4)])
            lgp = psM.tile([P, 4, 10], F32, tag="m", name="lgp")
            for e in range(4):
                nc.tensor.matmul(lgp[:, e, :], hhT[:, e, :], W["hW2"][:, e, :],
                                 start=True, stop=True)
            lgm = sm.tile([P, 4, 10], F32, tag="lgm", name="lgm")
            nc.vector.tensor_tensor(lgm[...], lgp[...],
                                    oh4f[:, :, None].broadcast_to([P, 4, 10]),
                                    ALU.mult)
            lgs = sm.tile([P, 10], F32, tag="lgs", name="lgs")
            nc.vector.tensor_reduce(lgs[...], lgm[...].rearrange("p e a -> p a e"),
                                    AX.X, ALU.add)
            if fl["hb2"]:
                hb2p = psM.tile([P, 10], F32, tag="m", name="hb2p")
                ohT4f = sm.tile([4, P], F32, tag="ohT4f", name="ohT4f")
                nc.scalar.copy(ohT4f[...], ohT4)
                nc.tensor.matmul(hb2p[...], ohT4f[...], W["hb2"][...],
                                 start=True, stop=True)
                nc.vector.tensor_tensor(lgs[...], lgs[...], hb2p[...], ALU.add)

            # ---- final masked softmax ----
            eL = sm.tile([P, 10], F32, tag="eL", name="eL")
            nc.scalar.activation(eL[...], lgs[...], AF.Exp)
            pm = sm.tile([P, 10], F32, tag="pm", name="pm")
            nc.vector.tensor_tensor(pm[...], eL[...], amask, ALU.mult)
            S1 = sm.tile([P, 1], F32, tag="S1", name="S1")
            nc.vector.tensor_reduce(S1[...], pm[...], AX.X, ALU.add)
            Sr = sm.tile([P, 1], F32, tag="Sr", name="Sr")
            nc.vector.reciprocal(Sr[...], S1[...])
            probs = io.tile([P, 10], F32, tag="probs", name="probs")
            nc.vector.tensor_scalar(probs[...], pm[...], Sr[...], None, ALU.mult)
            nc.sync.dma_start(probs_d[ts(t, P), :], probs[...])
            tile_scope.__exit__(None, None, None)


# ---------------------------------------------------------------- entry point

_CACHE = {}


def _get_kernel(n_tiles, consts, fl):
    key = n_tiles
    if key not in _CACHE:
        _CACHE[key] = build_kernel(n_tiles, consts, fl)
    return _CACHE[key]


def kernel(**inputs):
    params = inputs["params"]
    consts, fl = _prep_consts(params)
    n_tiles = B // N_CORES // P
    bc = n_tiles * P
    nc = _get_kernel(n_tiles, consts, fl)
    percore = _prep_percore(inputs, bc)
    in_maps = []
    for cc in range(N_CORES):
        m = dict(percore[cc])
        for name, arr in consts.items():
            m[name] = np.ascontiguousarray(arr)
        in_maps.append(m)
    res = bass_utils.run_bass_kernel_spmd(nc, in_maps, core_ids=list(range(N_CORES)))
    probs = np.concatenate([res.results[cc]["probs_out"] for cc in range(N_CORES)], axis=0)
    nctx = np.concatenate([res.results[cc]["nctx_out"] for cc in range(N_CORES)], axis=0)
    return probs, nctx


# revision 17
# speedup vs baseline: 1.0135x; 1.0135x over previous
"""Trainium2 Bass kernel for nn_DeepCFRModel (DeepCFR poker net).

Strategy: pure data-parallel over 8 NeuronCores (4096 rows each, 32 tiles of
128 rows on the partition dim). Host precomputes per-token-position embedding
tables (card/PE folded, QKV projected, biases folded) so all gathers become
one-hot matmuls on the PE. Attention head-reductions run on PE via stationary
elementwise-product tiles x block-ones; AV j-reduction via identity-matmul
PSUM accumulation. LayerNorm stats via bn_stats on row-major tiles.

All PE operands are bf16 (fp32 matmuls pay a ~4x stationary-load penalty);
PSUM accumulation stays fp32, as do LayerNorm stats, softmax, and the GRU
state arithmetic.
"""
import sys

if "/opt/trn_rl_repo" not in sys.path:
    sys.path.insert(0, "/opt/trn_rl_repo")

import numpy as np
import ml_dtypes

import concourse.bass as bass
import concourse.bacc as bacc
import concourse.mybir as mybir
import concourse.tile as tile
from concourse import bass_utils
from concourse.bass import ds, ts

F32 = mybir.dt.float32
BF16 = mybir.dt.bfloat16
AF = mybir.ActivationFunctionType
ALU = mybir.AluOpType
AX = mybir.AxisListType

P = 128
N_CORES = 8
B = 32768
D = 256
NH = 8
HD = 32
H = 256
EPS = 1e-5
BF = ml_dtypes.bfloat16


# ---------------------------------------------------------------- host prep

def _pe_table():
    pos = np.arange(5, dtype=np.float32)[:, None]
    div = np.exp(np.arange(0, D, 2, dtype=np.float32) * (-np.log(10000.0) / D))
    pe = np.zeros((5, D), np.float32)
    pe[:, 0::2] = np.sin(pos * div)
    pe[:, 1::2] = np.cos(pos * div)
    return pe


def _prep_consts(params):
    """Host-side packing of all weights/tables. Returns dict name -> np array
    (these become ExternalInputs, identical on every core) plus flag dict."""
    p = {k: np.asarray(v, np.float32) for k, v in params.items()}
    c = {}
    fl = {}

    # ---- token tables [53, 7, 256] (partition = token id) ----
    ct = p["card_table"]
    pe = _pe_table()
    xt = np.zeros((7, 53, D), np.float32)
    for t in range(7):
        xt[t, 1:] = ct
        if t >= 2:
            xt[t] += pe[t - 2][None, :]
    qt = (xt @ p["Wq"] + p["bq"]) / np.sqrt(HD)
    kt = xt @ p["Wk"] + p["bk"]
    vt = xt @ p["Wv"] + p["bv"]
    c["xtab"] = np.ascontiguousarray(xt.transpose(1, 0, 2)).astype(BF)
    c["qtab"] = np.ascontiguousarray(qt.transpose(1, 0, 2)).astype(BF)
    c["ktab"] = np.ascontiguousarray(kt.transpose(1, 0, 2)).astype(BF)
    c["vtab"] = np.ascontiguousarray(vt.transpose(1, 0, 2)).astype(BF)

    # ---- attention misc ----
    c["Wo"] = np.ascontiguousarray(
        p["Wo"].reshape(2, 128, 256).transpose(1, 0, 2)).astype(BF)
    fl["bo"] = bool(np.any(p["bo"]))
    c["bo"] = p["bo"].reshape(1, 256)
    fl["attn_gb"] = bool(np.any(p["attn_g"] != 1.0) or np.any(p["attn_b"]))
    c["attn_g"] = np.broadcast_to(p["attn_g"], (P, 256)).copy()
    c["attn_b"] = np.broadcast_to(p["attn_b"], (P, 256)).copy()

    # ---- f1 ----
    f1W = p["f1W"]
    c["f1Wc"] = np.ascontiguousarray(
        f1W[:1792].reshape(14, 128, 1024).transpose(1, 0, 2)).astype(BF)
    c["f1W9"] = f1W[1792:1801].astype(BF)                      # [9,1024]
    c["stage_f1"] = (p["stage_emb"] @ f1W[1801:1833]).astype(BF)   # [4,1024]
    c["pos_f1"] = (p["pos_emb"] @ f1W[1833:1865]).astype(BF)       # [3,1024]
    fl["f1b"] = bool(np.any(p["f1b"]))
    c["f1b"] = p["f1b"].reshape(1, 1024)
    fl["f1gb"] = bool(np.any(p["f1g"] != 1.0) or np.any(p["f1beta"]))
    c["f1g"] = np.broadcast_to(p["f1g"], (P, 1024)).copy()
    c["f1beta"] = np.broadcast_to(p["f1beta"], (P, 1024)).copy()

    # ---- f2, f3 ----
    c["f2W"] = np.ascontiguousarray(
        p["f2W"].reshape(8, 128, 512).transpose(1, 0, 2)).astype(BF)
    fl["f2b"] = bool(np.any(p["f2b"]))
    c["f2b"] = p["f2b"].reshape(1, 512)
    fl["f2gb"] = bool(np.any(p["f2g"] != 1.0) or np.any(p["f2beta"]))
    c["f2g"] = np.broadcast_to(p["f2g"], (P, 512)).copy()
    c["f2beta"] = np.broadcast_to(p["f2beta"], (P, 512)).copy()

    c["f3W"] = np.ascontiguousarray(
        p["f3W"].reshape(4, 128, 256).transpose(1, 0, 2)).astype(BF)
    fl["f3b"] = bool(np.any(p["f3b"]))
    c["f3b"] = p["f3b"].reshape(1, 256)
    fl["f3gb"] = bool(np.any(p["f3g"] != 1.0) or np.any(p["f3beta"]))
    c["f3g"] = np.broadcast_to(p["f3g"], (P, 256)).copy()
    c["f3beta"] = np.broadcast_to(p["f3beta"], (P, 256)).copy()

    # ---- GRU ----
    Wzr = np.concatenate([p["Wz"], p["Wr"]], axis=1)           # [512,512]
    c["Wzr"] = np.ascontiguousarray(
        Wzr.reshape(4, 128, 512).transpose(1, 0, 2)).astype(BF)
    fl["bzr"] = bool(np.any(p["bz"]) or np.any(p["br"]))
    c["bzr"] = np.concatenate([p["bz"], p["br"]]).reshape(1, 512)
    c["Wc"] = np.ascontiguousarray(
        p["Wc"].reshape(4, 128, 256).transpose(1, 0, 2)).astype(BF)
    fl["bc"] = bool(np.any(p["bc"]))
    c["bc"] = p["bc"].reshape(1, 256)

    # ---- main net ----
    m1Wf = p["m1W"] + p["m1skip"]
    c["m1W"] = np.ascontiguousarray(
        m1Wf.reshape(4, 128, 256).transpose(1, 0, 2)).astype(BF)
    fl["m1b"] = bool(np.any(p["m1b"]))
    c["m1b"] = p["m1b"].reshape(1, 256)
    fl["m1gb"] = bool(np.any(p["m1g"] != 1.0) or np.any(p["m1beta"]))
    c["m1g"] = np.broadcast_to(p["m1g"], (P, 256)).copy()
    c["m1beta"] = np.broadcast_to(p["m1beta"], (P, 256)).copy()

    c["m2W"] = np.ascontiguousarray(
        p["m2W"].reshape(2, 128, 256).transpose(1, 0, 2)).astype(BF)
    fl["m2b"] = bool(np.any(p["m2b"]))
    c["m2b"] = p["m2b"].reshape(1, 256)
    fl["m2gb"] = bool(np.any(p["m2g"] != 1.0) or np.any(p["m2beta"]))
    c["m2g"] = np.broadcast_to(p["m2g"], (P, 256)).copy()
    c["m2beta"] = np.broadcast_to(p["m2beta"], (P, 256)).copy()

    # ---- heads ----
    hW1 = p["hW1"]                                             # [4,256,128]
    hW1a = np.concatenate([hW1[e] for e in range(4)], axis=1)  # [256,512]
    c["hW1a"] = np.ascontiguousarray(
        hW1a.reshape(2, 128, 512).transpose(1, 0, 2)).astype(BF)
    fl["hb1"] = bool(np.any(p["hb1"]))
    hb1b = np.zeros((4, 512), np.float32)
    for e in range(4):
        hb1b[e, e * 128:(e + 1) * 128] = p["hb1"][e]
    c["hb1b"] = hb1b.astype(BF)
    c["hW2"] = np.ascontiguousarray(p["hW2"].transpose(1, 0, 2)).astype(BF)
    fl["hb2"] = bool(np.any(p["hb2"]))
    c["hb2"] = p["hb2"]                                        # [4,10] f32

    # ---- constants ----
    c["iota53"] = np.broadcast_to(np.arange(53, dtype=np.float32), (P, 53)).copy()
    c["iota4"] = np.broadcast_to(np.arange(4, dtype=np.float32), (P, 4)).copy()
    c["iota3"] = np.broadcast_to(np.arange(3, dtype=np.float32), (P, 3)).copy()
    c["identb"] = np.eye(P, dtype=np.float32).astype(BF)
    bones = np.zeros((128, 2, 8), np.float32)
    for half in range(2):
        for r in range(128):
            bones[r, half, (half * 128 + r) // 32] = 1.0
    c["bones"] = bones.astype(BF)
    c["ones1"] = np.ones((1, P), np.float32).astype(BF)
    c["epsv"] = np.full((P, 1), EPS, np.float32)

    # drop consts that the trivial-parameter specialization never reads
    drop = []
    if not fl["bo"]:
        drop.append("bo")
    if not fl["attn_gb"]:
        drop += ["attn_g", "attn_b"]
    for pref in ("f1", "f2", "f3", "m1", "m2"):
        if not fl[pref + "b"]:
            drop.append(pref + "b")
        if not fl[pref + "gb"]:
            drop += [pref + "g", pref + "beta"]
    if not fl["bzr"]:
        drop.append("bzr")
    if not fl["bc"]:
        drop.append("bc")
    if not fl["hb1"]:
        drop.append("hb1b")
    if not fl["hb2"]:
        drop.append("hb2")
    if not any(fl[k] for k in ("bo", "f1b", "f2b", "f3b", "bzr", "bc", "m1b", "m2b")):
        drop.append("ones1")
    for k in drop:
        c.pop(k, None)
    return c, fl


def _prep_percore(inputs, bc):
    """Per-core input arrays: misc [bc,28] and prevctx [bc,256]."""
    priv = np.asarray(inputs["private_cards"]).astype(np.float32) + 1.0
    pub = np.asarray(inputs["public_cards"]).astype(np.float32) + 1.0
    ids = np.concatenate([priv, pub], axis=1)                  # [B,7], 0=pad
    stage = np.asarray(inputs["stage"]).astype(np.float32)
    pos = np.asarray(inputs["current_player_pos"]).astype(np.float32)
    scal9 = np.concatenate([np.asarray(inputs["stacks"]),
                            np.asarray(inputs["bets"]),
                            np.asarray(inputs["active_players_mask"])],
                           axis=1).astype(np.float32)
    amask = np.asarray(inputs["actions_mask"]).astype(np.float32)
    misc = np.concatenate([ids, stage, pos, scal9, amask], axis=1)  # [B,28]
    pctx = np.asarray(inputs["prev_context"]).astype(np.float32)
    maps = []
    for cc in range(N_CORES):
        sl = slice(cc * bc, (cc + 1) * bc)
        maps.append({"misc": np.ascontiguousarray(misc[sl]),
                     "prevctx": np.ascontiguousarray(pctx[sl])})
    return maps


# ---------------------------------------------------------------- kernel build

def build_kernel(n_tiles, consts, fl):
    bc = n_tiles * P
    nc = bacc.Bacc("TRN2", target_bir_lowering=False, debug=False,
                   num_devices=N_CORES)

    dr = {}
    for name, arr in consts.items():
        dt = BF16 if arr.dtype == BF else F32
        dr[name] = nc.dram_tensor(name, list(arr.shape), dt, kind="ExternalInput")
    misc_d = nc.dram_tensor("misc", [bc, 28], F32, kind="ExternalInput")
    pctx_d = nc.dram_tensor("prevctx", [bc, 256], F32, kind="ExternalInput")
    probs_d = nc.dram_tensor("probs_out", [bc, 10], F32, kind="ExternalOutput")
    nctx_d = nc.dram_tensor("nctx_out", [bc, 256], F32, kind="ExternalOutput")

    with tile.TileContext(nc) as tc:
        _emit(nc, tc, dr, fl, misc_d, pctx_d, probs_d, nctx_d, n_tiles)
    nc.compile()
    return nc


def _emit(nc, tc, dr, fl, misc_d, pctx_d, probs_d, nctx_d, n_tiles):
    import contextlib
    ctx = contextlib.ExitStack()
    with ctx:
        # ---------------- weight pools (resident, bufs=1) ----------------
        wp = ctx.enter_context(tc.tile_pool(name="weights", bufs=1))
        W = {}
        for name, d in dr.items():
            t = wp.tile(list(d.shape), d.dtype, tag=name, name=name)
            nc.sync.dma_start(t[...], d[...])
            W[name] = t

        # ---------------- transient pools ----------------
        io = ctx.enter_context(tc.tile_pool(name="io", bufs=3))
        sm = ctx.enter_context(tc.tile_pool(name="small", bufs=2))
        mid = ctx.enter_context(tc.tile_pool(name="mid", bufs=2))
        mid2 = ctx.enter_context(tc.tile_pool(name="mid2", bufs=2))
        big = ctx.enter_context(tc.tile_pool(name="big", bufs=2))
        tpo = ctx.enter_context(tc.tile_pool(name="tpose", bufs=2))

        psBig = ctx.enter_context(tc.tile_pool(name="psBig", bufs=2, space="PSUM"))
        psS = ctx.enter_context(tc.tile_pool(name="psS", bufs=2, space="PSUM"))
        psM = ctx.enter_context(tc.tile_pool(name="psM", bufs=2, space="PSUM"))

        identb = W["identb"]

        def transpose_pack(out_sb, srcs):
            """PE-transpose each [128,128-or-less] bf16 src into one shared
            PSUM tile (all fit one bank), then one ACT copy to out_sb (bf16).
            srcs: list of APs, each [128, w]; out_sb [sum-w?, k, 128]-ish AP
            arranged [w, k, 128] written slice per src."""
            k = len(srcs)
            w = max(s.shape[1] for s in srcs)
            pst = psM.tile([w, k, P], BF16, tag="m", name="pstT")
            for idx, s in enumerate(srcs):
                nc.tensor.matmul(pst[0:s.shape[1], idx, :], s, identb[...],
                                 is_transpose=True, start=True, stop=True)
            nc.scalar.copy(out_sb, pst[...])

        def ln_apply(z_ps, width, pref, gelu, out_tile):
            """Full-width LN over `width` (+ optional gelu) PSUM z -> SBUF."""
            csz = min(width, 512)
            chunks = width // csz
            stats = sm.tile([P, chunks, 6], F32, tag="stats", name="stats")
            zc = z_ps.rearrange("p (c f) -> p c f", c=chunks)
            for ci in range(chunks):
                nc.vector.bn_stats(stats[:, ci, :], zc[:, ci, :])
            mv = sm.tile([P, 2], F32, tag="mv", name="mv")
            nc.vector.bn_aggr(mv[...], stats[...])
            sd = sm.tile([P, 1], F32, tag="sd", name="sd")
            nc.scalar.activation(sd[...], mv[:, 1:2], AF.Sqrt, bias=W["epsv"][...])
            rstd = sm.tile([P, 1], F32, tag="rstd", name="rstd")
            nc.vector.reciprocal(rstd[...], sd[...])
            if fl[pref + "gb"]:
                t0 = mid.tile([P, width], F32, tag="lnt0", name="lnt0")
                nc.vector.tensor_scalar(t0[...], z_ps, mv[:, 0:1], rstd[...],
                                        ALU.subtract, ALU.mult)
                t1 = mid.tile([P, width], F32, tag="lnt1", name="lnt1")
                nc.vector.tensor_tensor(t1[...], t0[...], W[pref + "g"][...], ALU.mult)
                pre = mid.tile([P, width], F32, tag="lnt2", name="lnt2")
                nc.vector.tensor_tensor(pre[...], t1[...], W[pref + "beta"][...], ALU.add)
            else:
                pre = mid.tile([P, width], F32, tag="lnt0", name="lnt0")
                nc.vector.tensor_scalar(pre[...], z_ps, mv[:, 0:1], rstd[...],
                                        ALU.subtract, ALU.mult)
            if gelu:
                nc.scalar.activation(out_tile[...], pre[...], AF.Gelu)
            else:
                nc.scalar.copy(out_tile[...], pre[...])

        def ln_apply_2ps(z_halves, width, pref, gelu, out_tile):
            """LN over `width` spread across two PSUM half-tiles."""
            hw = width // 2
            chunks_per = max(1, hw // 512)
            csz = hw // chunks_per
            nch = 2 * chunks_per
            stats = sm.tile([P, nch, 6], F32, tag="stats", name="stats2")
            for hi in range(2):
                zc = z_halves[hi][...].rearrange("p (c f) -> p c f", c=chunks_per)
                for ci in range(chunks_per):
                    nc.vector.bn_stats(stats[:, hi * chunks_per + ci, :], zc[:, ci, :])
            mv = sm.tile([P, 2], F32, tag="mv", name="mv2")
            nc.vector.bn_aggr(mv[...], stats[...])
            sd = sm.tile([P, 1], F32, tag="sd", name="sd2")
            nc.scalar.activation(sd[...], mv[:, 1:2], AF.Sqrt, bias=W["epsv"][...])
            rstd = sm.tile([P, 1], F32, tag="rstd", name="rstd2")
            nc.vector.reciprocal(rstd[...], sd[...])
            for hi in range(2):
                if fl[pref + "gb"]:
                    t0 = mid.tile([P, hw], F32, tag="lnt0", name="lnt0h")
                    nc.vector.tensor_scalar(t0[...], z_halves[hi][...],
                                            mv[:, 0:1], rstd[...],
                                            ALU.subtract, ALU.mult)
                    t1 = mid.tile([P, hw], F32, tag="lnt1", name="lnt1h")
                    nc.vector.tensor_tensor(
                        t1[...], t0[...], W[pref + "g"][:, ds(hi * hw, hw)], ALU.mult)
                    pre = mid.tile([P, hw], F32, tag="lnt2", name="lnt2h")
                    nc.vector.tensor_tensor(
                        pre[...], t1[...], W[pref + "beta"][:, ds(hi * hw, hw)], ALU.add)
                else:
                    pre = mid.tile([P, hw], F32, tag="lnt0", name="lnt0h")
                    nc.vector.tensor_scalar(pre[...], z_halves[hi][...],
                                            mv[:, 0:1], rstd[...],
                                            ALU.subtract, ALU.mult)
                if gelu:
                    nc.scalar.activation(out_tile[:, ds(hi * hw, hw)], pre[...],
                                         AF.Gelu)
                else:
                    nc.scalar.copy(out_tile[:, ds(hi * hw, hw)], pre[...])

        def bias_mm(ps_ap, bias_name, n0, n1, stop):
            nc.tensor.matmul(ps_ap, W["ones1"][0:1, :], W[bias_name][0:1, n0:n1],
                             start=False, stop=stop)

        # ================= per-tile loop (2-stage software pipeline) =================
        state = {}

        def emit_A(t):
            st = {}
            state[t] = st
            # ---- input DMA ----
            misc = io.tile([P, 28], F32, tag="misc", name="misc")
            nc.sync.dma_start(misc[...], misc_d[ts(t, P), :])
            pctx = io.tile([P, 256], F32, tag="pctx", name="pctx")
            nc.sync.dma_start(pctx[...], pctx_d[ts(t, P), :])
            ids = misc[:, 0:7]
            stagef = misc[:, 7:8]
            posf = misc[:, 8:9]
            scal9 = misc[:, 9:18]
            amask = misc[:, 18:28]

            # ---- one-hots (DVE; bf16 out, values 0/1 exact) ----
            oh = mid2.tile([P, 7, 53], BF16, tag="oh", name="oh")
            nc.vector.tensor_tensor(
                oh[...], ids[:, :, None].broadcast_to([P, 7, 53]),
                W["iota53"][:, None, :].broadcast_to([P, 7, 53]), ALU.is_equal)
            oh4 = sm.tile([P, 4], BF16, tag="oh4", name="oh4")
            nc.vector.tensor_tensor(oh4[...], stagef.broadcast_to([P, 4]),
                                    W["iota4"][...], ALU.is_equal)
            oh4f = sm.tile([P, 4], F32, tag="oh4f", name="oh4f")
            nc.vector.tensor_tensor(oh4f[...], stagef.broadcast_to([P, 4]),
                                    W["iota4"][...], ALU.is_equal)
            oh3 = sm.tile([P, 3], BF16, tag="oh3", name="oh3")
            nc.vector.tensor_tensor(oh3[...], posf.broadcast_to([P, 3]),
                                    W["iota3"][...], ALU.is_equal)
            s9b = sm.tile([P, 9], BF16, tag="s9b", name="s9b")
            nc.scalar.copy(s9b[...], scal9)
            pctxb = sm.tile([P, 256], BF16, tag="pctxb", name="pctxb")
            nc.scalar.copy(pctxb[...], pctx[...])
            mb = sm.tile([P, 7], F32, tag="mb", name="mb")
            nc.vector.tensor_scalar(mb[...], ids, 0.0, -1e9, ALU.is_equal, ALU.mult)

            # ---- transposes of onehots / small inputs (packed) ----
            ohT = mid2.tile([53, 7, P], BF16, tag="ohT", name="ohT")
            transpose_pack(ohT[:, 0:4, :], [oh[:, tok, :] for tok in range(4)])
            transpose_pack(ohT[:, 4:7, :], [oh[:, tok, :] for tok in range(4, 7)])
            smT = tpo.tile([9, 3, P], BF16, tag="smT", name="smT")
            transpose_pack(smT[...], [oh4[...], oh3[...], s9b[...]])
            ohT4, ohT3, s9T = smT[0:4, 0, :], smT[0:3, 1, :], smT[0:9, 2, :]
            pctxT = tpo.tile([P, 2, P], BF16, tag="pctxT", name="pctxT")
            transpose_pack(pctxT[...], [pctxb[:, ds(c * P, P)] for c in range(2)])

            # ---- phase A: q,k FM halves + scores (accumulate across halves) ----
            sp = psS.tile([P, 7, 7, 8], F32, tag="s", name="sp")
            for half in range(2):
                qp = psBig.tile([P, 7, P], F32, tag="big", name="qp")
                kp = psBig.tile([P, 7, P], F32, tag="big", name="kp")
                for tok in range(7):
                    nc.tensor.matmul(qp[:, tok, :],
                                     W["qtab"][:, tok, ds(half * P, P)],
                                     ohT[:, tok, :], start=True, stop=True)
                    nc.tensor.matmul(kp[:, tok, :],
                                     W["ktab"][:, tok, ds(half * P, P)],
                                     ohT[:, tok, :], start=True, stop=True)
                q_sb = mid2.tile([P, 7, P], BF16, tag="q_sb", name="q_sb")
                nc.scalar.copy(q_sb[...], qp[...])
                k_sb = mid2.tile([P, 7, P], BF16, tag="k_sb", name="k_sb")
                nc.scalar.copy(k_sb[...], kp[...])
                for i in range(7):
                    e_sb = mid2.tile([P, 7, P], BF16, tag="e", name="e")
                    nc.vector.tensor_tensor(
                        e_sb[...], q_sb[:, i:i + 1, :].broadcast_to([P, 7, P]),
                        k_sb[...], ALU.mult)
                    for j in range(7):
                        nc.tensor.matmul(sp[:, i, j, :], e_sb[:, j, :],
                                         W["bones"][:, half, :],
                                         start=(half == 0), stop=(half == 1))

            st.update(misc=misc, pctx=pctx, oh4f=oh4f, mb=mb, ohT=ohT,
                      smT=smT, pctxT=pctxT, sp=sp)

        def emit_rest(t):
            st = state.pop(t)
            misc, pctx, oh4f, mb = st["misc"], st["pctx"], st["oh4f"], st["mb"]
            ohT, smT, pctxT, sp = st["ohT"], st["smT"], st["pctxT"], st["sp"]
            ohT4, ohT3, s9T = smT[0:4, 0, :], smT[0:3, 1, :], smT[0:9, 2, :]
            amask = misc[:, 18:28]
            # ---- softmax (row-major) ----
            smx = mid.tile([P, 7, 7, 8], F32, tag="smx", name="smx")
            nc.vector.tensor_tensor(
                smx[...], sp[...],
                mb[:, None, :, None].broadcast_to([P, 7, 7, 8]), ALU.add)
            pex = mid.tile([P, 7, 7, 8], F32, tag="pex", name="pex")
            nc.scalar.activation(pex[...], smx[...], AF.Exp)
            zs = sm.tile([P, 7, 8], F32, tag="zs", name="zs")
            nc.vector.tensor_reduce(zs[...], pex[...].rearrange("p i j h -> p i h j"),
                                    AX.X, ALU.add)
            zr = sm.tile([P, 7, 8], F32, tag="zr", name="zr")
            nc.vector.reciprocal(zr[...], zs[...])
            pn = mid.tile([P, 7, 7, 8], BF16, tag="pn", name="pn")
            nc.vector.tensor_tensor(
                pn[...], pex[...],
                zr[:, :, None, :].broadcast_to([P, 7, 7, 8]), ALU.mult)

            # ---- phase B: v + AV ----
            v03 = psBig.tile([P, 4, 256], F32, tag="big", name="v03")
            v46 = psBig.tile([P, 3, 256], F32, tag="big", name="v46")
            for tok in range(7):
                vdst = v03[:, tok, :] if tok < 4 else v46[:, tok - 4, :]
                nc.tensor.matmul(vdst, ohT[:, tok, :], W["vtab"][:, tok, :],
                                 start=True, stop=True)
            v_sb = big.tile([P, 7, 256], BF16, tag="v_sb", name="v_sb")
            nc.scalar.copy(v_sb[:, 0:4, :], v03[...])
            nc.scalar.copy(v_sb[:, 4:7, :], v46[...])
            o_sb = big.tile([P, 7, 256], BF16, tag="o", name="o_sb")
            for i in range(7):
                pv = big.tile([P, 7, 8, HD], BF16, tag="pv", name="pv")
                nc.vector.tensor_tensor(
                    pv[...],
                    pn[:, i, :, :, None].broadcast_to([P, 7, 8, HD]),
                    v_sb[...].rearrange("p j (h d) -> p j h d", h=8), ALU.mult)
                op = psM.tile([P, 256], F32, tag="m", name="op")
                for j in range(7):
                    nc.tensor.matmul(op[...], identb[...],
                                     pv[:, j, :, :].rearrange("p h d -> p (h d)"),
                                     start=(j == 0), stop=(j == 6))
                nc.scalar.copy(o_sb[:, i, :], op[...])

            # ---- phase C: o-proj + x + per-token LN ----
            cp03 = psBig.tile([P, 4, 256], F32, tag="big", name="cp03")
            cp46 = psBig.tile([P, 3, 256], F32, tag="big", name="cp46")
            for tok in range(7):
                cdst = cp03[:, tok, :] if tok < 4 else cp46[:, tok - 4, :]
                oT = tpo.tile([P, 2, P], BF16, tag="oT", name="oT")
                transpose_pack(oT[...], [o_sb[:, tok, ds(c * P, P)] for c in range(2)])
                nc.tensor.matmul(cdst, oT[:, 0, :], W["Wo"][:, 0, :],
                                 start=True, stop=False)
                nc.tensor.matmul(cdst, oT[:, 1, :], W["Wo"][:, 1, :],
                                 start=False, stop=False)
                nc.tensor.matmul(cdst, ohT[:, tok, :], W["xtab"][:, tok, :],
                                 start=False, stop=not fl["bo"])
                if fl["bo"]:
                    bias_mm(cdst, "bo", 0, 256, True)

            cf = big.tile([P, 7, 256], BF16, tag="cf", name="cf")
            for grp, cpt, ntok in ((0, cp03, 4), (4, cp46, 3)):
                stats = sm.tile([P, ntok, 6], F32, tag="stats", name="statsA")
                for tt in range(ntok):
                    nc.vector.bn_stats(stats[:, tt, :], cpt[:, tt, :])
                for tt in range(ntok):
                    tok = grp + tt
                    mv = sm.tile([P, 2], F32, tag="mv", name="mvA")
                    nc.vector.bn_aggr(mv[...], stats[:, tt, :])
                    sd = sm.tile([P, 1], F32, tag="sd", name="sdA")
                    nc.scalar.activation(sd[...], mv[:, 1:2], AF.Sqrt,
                                         bias=W["epsv"][...])
                    rstd = sm.tile([P, 1], F32, tag="rstd", name="rstdA")
                    nc.vector.reciprocal(rstd[...], sd[...])
                    if fl["attn_gb"]:
                        cfw = mid.tile([P, 256], F32, tag="cfw", name="cfw")
                        nc.vector.tensor_scalar(cfw[...], cpt[:, tt, :],
                                                mv[:, 0:1], rstd[...],
                                                ALU.subtract, ALU.mult)
                        cfw2 = mid.tile([P, 256], F32, tag="cfw2", name="cfw2")
                        nc.vector.tensor_tensor(cfw2[...], cfw[...],
                                                W["attn_g"][...], ALU.mult)
                        nc.vector.tensor_tensor(cf[:, tok, :], cfw2[...],
                                                W["attn_b"][...], ALU.add)
                    else:
                        nc.vector.tensor_scalar(cf[:, tok, :], cpt[:, tt, :],
                                                mv[:, 0:1], rstd[...],
                                                ALU.subtract, ALU.mult)

            # ---- f1 (one PSUM bank at a time, raw halves staged in SBUF) ----
            cfT = mid.tile([P, 14, P], BF16, tag="cfT", name="cfT")
            cfv = cf[...].rearrange("p t d -> p (t d)")
            for kb2 in range(7):
                transpose_pack(cfT[:, ds(2 * kb2, 2), :],
                               [cfv[:, ds((2 * kb2 + c) * P, P)] for c in range(2)])
            h1raw = mid.tile([P, 1024], F32, tag="h1raw", name="h1raw")
            for nh2 in range(2):
                h1p = psM.tile([P, 512], F32, tag="m", name="h1p")
                nsl = ds(nh2 * 512, 512)
                for kb in range(14):
                    nc.tensor.matmul(h1p[...], cfT[:, kb, :],
                                     W["f1Wc"][:, kb, nsl],
                                     start=(kb == 0), stop=False)
                nc.tensor.matmul(h1p[...], s9T, W["f1W9"][:, nsl],
                                 start=False, stop=False)
                nc.tensor.matmul(h1p[...], ohT4, W["stage_f1"][:, nsl],
                                 start=False, stop=False)
                nc.tensor.matmul(h1p[...], ohT3, W["pos_f1"][:, nsl],
                                 start=False, stop=not fl["f1b"])
                if fl["f1b"]:
                    bias_mm(h1p[...], "f1b", nh2 * 512, nh2 * 512 + 512, True)
                nc.scalar.copy(h1raw[:, nsl], h1p[...])
            h1 = mid.tile([P, 1024], BF16, tag="h1", name="h1")
            ln_apply(h1raw[...], 1024, "f1", True, h1)

            # ---- f2 ----
            h2p = psM.tile([P, 512], F32, tag="m", name="h2p")
            for kb2 in range(4):
                h1T = tpo.tile([P, 2, P], BF16, tag="h1T", name="h1T")
                transpose_pack(h1T[...],
                               [h1[:, ds((2 * kb2 + c) * P, P)] for c in range(2)])
                for c in range(2):
                    kb = 2 * kb2 + c
                    nc.tensor.matmul(h2p[...], h1T[:, c, :], W["f2W"][:, kb, :],
                                     start=(kb == 0),
                                     stop=(kb == 7) and not fl["f2b"])
            if fl["f2b"]:
                bias_mm(h2p[...], "f2b", 0, 512, True)
            h2 = mid.tile([P, 512], BF16, tag="h2", name="h2")
            ln_apply(h2p[...], 512, "f2", True, h2)

            # ---- f3 ----
            f3p = psM.tile([P, 256], F32, tag="m", name="f3p")
            h2T = tpo.tile([P, 4, P], BF16, tag="h2T", name="h2T")
            transpose_pack(h2T[...], [h2[:, ds(c * P, P)] for c in range(4)])
            for kb in range(4):
                nc.tensor.matmul(f3p[...], h2T[:, kb, :], W["f3W"][:, kb, :],
                                 start=(kb == 0), stop=(kb == 3) and not fl["f3b"])
            if fl["f3b"]:
                bias_mm(f3p[...], "f3b", 0, 256, True)
            feats = io.tile([P, 256], BF16, tag="feats", name="feats")
            ln_apply(f3p[...], 256, "f3", True, feats)

            # ---- GRU ----
            featsT = tpo.tile([P, 2, P], BF16, tag="featsT", name="featsT")
            transpose_pack(featsT[...], [feats[:, ds(c * P, P)] for c in range(2)])
            zrp = psM.tile([P, 512], F32, tag="m", name="zrp")
            combT = [featsT[:, 0, :], featsT[:, 1, :], pctxT[:, 0, :], pctxT[:, 1, :]]
            for kb in range(4):
                nc.tensor.matmul(zrp[...], combT[kb], W["Wzr"][:, kb, :],
                                 start=(kb == 0), stop=(kb == 3) and not fl["bzr"])
            if fl["bzr"]:
                bias_mm(zrp[...], "bzr", 0, 512, True)
            zr_sb = mid.tile([P, 512], F32, tag="zrg", name="zr_sb")
            nc.scalar.activation(zr_sb[...], zrp[...], AF.Sigmoid)
            rp = sm.tile([P, 256], BF16, tag="rp", name="rp")
            nc.vector.tensor_tensor(rp[...], zr_sb[:, 256:512], pctx[...], ALU.mult)
            rpT = tpo.tile([P, 2, P], BF16, tag="rpT", name="rpT")
            transpose_pack(rpT[...], [rp[:, ds(c * P, P)] for c in range(2)])
            cdp = psM.tile([P, 256], F32, tag="m", name="cdp")
            candT = [featsT[:, 0, :], featsT[:, 1, :], rpT[:, 0, :], rpT[:, 1, :]]
            for kb in range(4):
                nc.tensor.matmul(cdp[...], candT[kb], W["Wc"][:, kb, :],
                                 start=(kb == 0), stop=(kb == 3) and not fl["bc"])
            if fl["bc"]:
                bias_mm(cdp[...], "bc", 0, 256, True)
            cand = sm.tile([P, 256], F32, tag="cand", name="cand")
            nc.scalar.activation(cand[...], cdp[...], AF.Tanh)
            t1 = sm.tile([P, 256], F32, tag="t1", name="t1")
            nc.vector.tensor_tensor(t1[...], cand[...], pctx[...], ALU.subtract)
            t2 = sm.tile([P, 256], F32, tag="t2", name="t2")
            nc.vector.tensor_tensor(t2[...], zr_sb[:, 0:256], t1[...], ALU.mult)
            nctx = io.tile([P, 256], F32, tag="nctx", name="nctx")
            nc.vector.tensor_tensor(nctx[...], pctx[...], t2[...], ALU.add)
            nc.sync.dma_start(nctx_d[ts(t, P), :], nctx[...])
            nctxb = sm.tile([P, 256], BF16, tag="nctxb", name="nctxb")
            nc.scalar.copy(nctxb[...], nctx[...])

            # ---- m1 ----
            nctxT = tpo.tile([P, 2, P], BF16, tag="nctxT", name="nctxT")
            transpose_pack(nctxT[...], [nctxb[:, ds(c * P, P)] for c in range(2)])
            m1p = psM.tile([P, 256], F32, tag="m", name="m1p")
            mcT = [featsT[:, 0, :], featsT[:, 1, :], nctxT[:, 0, :], nctxT[:, 1, :]]
            for kb in range(4):
                nc.tensor.matmul(m1p[...], mcT[kb], W["m1W"][:, kb, :],
                                 start=(kb == 0), stop=(kb == 3) and not fl["m1b"])
            if fl["m1b"]:
                bias_mm(m1p[...], "m1b", 0, 256, True)
            y1 = sm.tile([P, 256], BF16, tag="y1", name="y1")
            ln_apply(m1p[...], 256, "m1", True, y1)

            # ---- m2 (+residual) ----
            m2p = psM.tile([P, 256], F32, tag="m", name="m2p")
            y1T = tpo.tile([P, 2, P], BF16, tag="y1T", name="y1T")
            transpose_pack(y1T[...], [y1[:, ds(c * P, P)] for c in range(2)])
            for kb in range(2):
                nc.tensor.matmul(m2p[...], y1T[:, kb, :], W["m2W"][:, kb, :],
                                 start=(kb == 0), stop=False)
            nc.tensor.matmul(m2p[...], identb[...], y1[...],
                             start=False, stop=not fl["m2b"])
            if fl["m2b"]:
                bias_mm(m2p[...], "m2b", 0, 256, True)
            y2 = sm.tile([P, 256], BF16, tag="y2", name="y2")
            ln_apply(m2p[...], 256, "m2", True, y2)

            # ---- heads ----
            hhp = psM.tile([P, 512], F32, tag="m", name="hhp")
            y2T = tpo.tile([P, 2, P], BF16, tag="y2T", name="y2T")
            transpose_pack(y2T[...], [y2[:, ds(c * P, P)] for c in range(2)])
            for kb in range(2):
                nc.tensor.matmul(hhp[...], y2T[:, kb, :], W["hW1a"][:, kb, :],
                                 start=(kb == 0), stop=(kb == 1) and not fl["hb1"])
            if fl["hb1"]:
                nc.tensor.matmul(hhp[...], ohT4, W["hb1b"][...],
                                 start=False, stop=True)
            hh = mid.tile([P, 512], BF16, tag="hh", name="hh")
            nc.scalar.activation(hh[...], hhp[...], AF.Gelu)
            hhT = tpo.tile([P, 4, P], BF16, tag="hhT", name="hhT")
            transpose_pack(hhT[...], [hh[:, ds(e * P, P)] for e in range(4)])
            lgp = psM.tile([P, 4, 10], F32, tag="m", name="lgp")
            for e in range(4):
                nc.tensor.matmul(lgp[:, e, :], hhT[:, e, :], W["hW2"][:, e, :],
                                 start=True, stop=True)
            lgm = sm.tile([P, 4, 10], F32, tag="lgm", name="lgm")
            nc.vector.tensor_tensor(lgm[...], lgp[...],
                                    oh4f[:, :, None].broadcast_to([P, 4, 10]),
                                    ALU.mult)
            lgs = sm.tile([P, 10], F32, tag="lgs", name="lgs")
            nc.vector.tensor_reduce(lgs[...], lgm[...].rearrange("p e a -> p a e"),
                                    AX.X, ALU.add)
            if fl["hb2"]:
                hb2p = psM.tile([P, 10], F32, tag="m", name="hb2p")
                ohT4f = sm.tile([4, P], F32, tag="ohT4f", name="ohT4f")
                nc.scalar.copy(ohT4f[...], ohT4)
                nc.tensor.matmul(hb2p[...], ohT4f[...], W["hb2"][...],
                                 start=True, stop=True)
                nc.vector.tensor_tensor(lgs[...], lgs[...], hb2p[...], ALU.add)

            # ---- final masked softmax ----
            eL = sm.tile([P, 10], F32, tag="eL", name="eL")
            nc.scalar.activation(eL[...], lgs[...], AF.Exp)
            pm = sm.tile([P, 10], F32, tag="pm", name="pm")
            nc.vector.tensor_tensor(pm[...], eL[...], amask, ALU.mult)
            S1 = sm.tile([P, 1], F32, tag="S1", name="S1")
            nc.vector.tensor_reduce(S1[...], pm[...], AX.X, ALU.add)
            Sr = sm.tile([P, 1], F32, tag="Sr", name="Sr")
            nc.vector.reciprocal(Sr[...], S1[...])
            probs = io.tile([P, 10], F32, tag="probs", name="probs")
            nc.vector.tensor_scalar(probs[...], pm[...], Sr[...], None, ALU.mult)
            nc.sync.dma_start(probs_d[ts(t, P), :], probs[...])

        for t in range(n_tiles + 1):
            if t < n_tiles:
                with nc.named_scope(f"A{t:02d}"):
                    emit_A(t)
            if t >= 1:
                with nc.named_scope(f"R{t - 1:02d}"):
                    emit_rest(t - 1)


# ---------------------------------------------------------------- entry point

_CACHE = {}


def _get_kernel(n_tiles, consts, fl):
    key = n_tiles
    if key not in _CACHE:
        _CACHE[key] = build_kernel(n_tiles, consts, fl)
    return _CACHE[key]


def kernel(**inputs):
    params = inputs["params"]
    consts, fl = _prep_consts(params)
    n_tiles = B // N_CORES // P
    bc = n_tiles * P
    nc = _get_kernel(n_tiles, consts, fl)
    percore = _prep_percore(inputs, bc)
    in_maps = []
    for cc in range(N_CORES):
        m = dict(percore[cc])
        for name, arr in consts.items():
            m[name] = np.ascontiguousarray(arr)
        in_maps.append(m)
    res = bass_utils.run_bass_kernel_spmd(nc, in_maps, core_ids=list(range(N_CORES)))
    probs = np.concatenate([res.results[cc]["probs_out"] for cc in range(N_CORES)], axis=0)
    nctx = np.concatenate([res.results[cc]["nctx_out"] for cc in range(N_CORES)], axis=0)
    return probs, nctx
